# revision 39
# baseline (speedup 1.0000x reference)
"""DKF (deep Kalman filter) Trainium2 kernel.

Self-contained: takes FULL inputs (xs [512,1000,64], eps [512,1000,10],
mask [512,1000] bool, params dict), returns (z, kl, iwae) batch-major,
matching the jax reference. Shards batch 512 -> 8 cores x 64.

Per core, one Tile program:
  phase 1 (serial scan over T=1000): z_t = qmu + softplus(s)*eps via
    feature-major matmuls (batch on the free axis), softplus composed as
    relu(s) + u*Q(u), u = exp(-|s|) (deg-7 poly, custom DVE ops) so the
    whole kernel stays inside the exp_and_others ACT table set.
  phase 2 (wide, overlapped): prior nets / kl / iwae recomputed for all t
    in a pack-4 layout (t%4 -> partition base 32j) with block-diagonal
    matmuls; ln(sigma) via a deg-15 poly of ln(softplus(s)) on DVE
    (avoids ACT table switches); PE transposes to batch-major outputs.
"""

import numpy as np

LATENT_D, GT_SIZE, EZ_SIZE, NW = 10, 64, 32, 10
B_FULL, T_FULL = 512, 1000
N_CORES = 8
B_SH = B_FULL // N_CORES  # 64 batch per core

_HALF_LOG_2PI = 0.9189385332046727


# ---------------------------------------------------------------- numerics --
def _fit_cheb(fn, deg, a, b, N=40001):
    x = (np.cos(np.pi * (np.arange(N) + 0.5) / N)[::-1]) * (b - a) / 2 + (a + b) / 2
    V = np.polynomial.chebyshev.chebvander((2 * x - (a + b)) / (b - a), deg)
    c, *_ = np.linalg.lstsq(V, fn(x), rcond=None)
    return np.polynomial.chebyshev.Chebyshev(c, domain=[a, b]).convert(
        kind=np.polynomial.Polynomial
    ).coef


# softplus(s) = relu(s) + u*Q(u), u = exp(-|s|): Q deg-5 of log1p(u)/u on (0,1]
_SP_C = _fit_cheb(lambda u: np.log1p(u) / u, 5, 1e-9, 1.0)
# g(s) = ln(softplus(s)) deg-14 on [-4.5, 4.5]
_G_C = _fit_cheb(lambda s: np.log(np.logaddexp(0, s)), 14, -4.5, 4.5)


# ------------------------------------------------------------ numpy fallback --
def _numpy_ref(xs, eps, mask, P):
    B, T, _ = xs.shape
    sp = lambda x: np.logaddexp(0, x)
    sig = lambda x: 0.5 * (np.tanh(0.5 * x) + 1.0)
    nlp = lambda x, m, s: -0.5 * np.square((x - m) / s) - np.log(s) - _HALF_LOG_2PI
    z = np.zeros((B, LATENT_D), np.float64)
    zs = np.empty((B, T, LATENT_D), np.float32)
    kls = np.empty((B, T, LATENT_D), np.float32)
    iws = np.empty((B, T), np.float32)
    for t in range(T):
        xt = 0.5 * (np.tanh(z @ P['gt'][0] + P['gt'][1]) + xs[:, t])
        xt = np.tanh(xt @ P['ez'][0] + P['ez'][1])
        q_mu = xt @ P['mu'][0] + P['mu'][1]
        q_sigma = sp(xt @ P['sigma'][0] + P['sigma'][1])
        nu = sig(np.maximum(z @ P['nu1'][0] + P['nu1'][1], 0) @ P['nu2'][0] + P['nu2'][1])
        mun = np.maximum(z @ P['mun1'][0] + P['mun1'][1], 0) @ P['mun2'][0] + P['mun2'][1]
        mul = z @ P['mul'][0] + P['mul'][1]
        p_mu = (1.0 - nu) * mul + nu * mun
        p_sigma = sp(np.maximum(mun, 0) @ P['dzsig'][0] + P['dzsig'][1])
        kl = (np.log(p_sigma / q_sigma)
              + (np.square(q_sigma) + np.square(q_mu - p_mu)) / (2.0 * np.square(p_sigma))
              - 0.5)
        e = eps[:, t]
        new = q_sigma * e + q_mu
        pz_s = np.clip(p_sigma * e + p_mu, -1e6, 1e6)
        new = np.where(mask[:, t][:, None], new, pz_s)
        iwae = (nlp(new, q_mu, q_sigma) - nlp(new, p_mu, p_sigma)).sum(-1)
        zs[:, t], kls[:, t], iws[:, t] = new, kl, iwae
        z = new
    return zs, kls, iws


# ------------------------------------------------------- custom DVE ops -----
_OPS = {}


def _register_ops():
    if _OPS:
        return _OPS
    import concourse.dve_ops as dvo
    from concourse.dve_spec import (
        Spec, Src0, Src1, C0, C1, C2, Zero, maxx, minn, lower, _has_src1,
    )
    from concourse.dve_uop import DveOpSpec

    def reg(name, spec):
        for o in dvo.OPS:
            if o.name == name:
                return o
        uops = lower(spec, ver="v3")
        sha = DveOpSpec(name=name, opcode=1, uops=uops, rd1_en=_has_src1(spec)).sha("v3")
        op = dvo.DveOp(name, spec, subdim=False, uops_sha={"v3": sha})
        dvo.OPS.append(op)
        dvo.CUSTOM_DVE_SPECS[name] = spec
        dvo._SUB_OPCODE_FOR_NAME[name] = dvo._CUSTOM_DVE_ROW_BASE + len(dvo.OPS) - 1
        return op

    f32 = np.float32
    _OPS['MINNEG'] = reg("DKF_MINNEG", Spec(
        body=minn(Src0, Zero - Src0),
        reference=lambda in0, in1, s0, s1, imm2: np.minimum(in0, -in0).astype(f32)))
    _OPS['SEED3'] = reg("DKF_SEED3", Spec(
        body=(C0 * Src0 + C1) * Src0 + C2,
        reference=lambda in0, in1, s0, s1, imm2: ((s0 * in0 + s1) * in0 + imm2).astype(f32)))
    _OPS['CHAIN3'] = reg("DKF_CHAIN3", Spec(
        body=((Src1 * Src0 + C0) * Src0 + C1) * Src0 + C2,
        reference=lambda in0, in1, s0, s1, imm2: (((in1 * in0 + s0) * in0 + s1) * in0 + imm2).astype(f32)))
    _OPS['CHAIN1M'] = reg("DKF_CHAIN1M", Spec(
        body=(Src1 * Src0 + C0) * Src0,
        reference=lambda in0, in1, s0, s1, imm2: ((in1 * in0 + s0) * in0).astype(f32)))
    _OPS['CHAIN3M'] = reg("DKF_CHAIN3M", Spec(
        body=(((Src1 * Src0 + C0) * Src0 + C1) * Src0 + C2) * Src0,
        reference=lambda in0, in1, s0, s1, imm2:
            ((((in1 * in0 + s0) * in0 + s1) * in0 + imm2) * in0).astype(f32)))
    _OPS['CHAIN1'] = reg("DKF_CHAIN1", Spec(
        body=Src1 * Src0 + C0,
        reference=lambda in0, in1, s0, s1, imm2: (in1 * in0 + s0).astype(f32)))
    _OPS['RELUADD'] = reg("DKF_RELUADD", Spec(
        body=maxx(Src0, Zero) + Src1,
        reference=lambda in0, in1, s0, s1, imm2: (np.maximum(in0, 0) + in1).astype(f32)))
    return _OPS


def _emit_softplus(nc, out, sb, u, t1, t2):
    """out = relu(sb) + u*Q(u); u = exp(-|sb|) already computed."""
    q = [float(v) for v in _SP_C]
    nc.vector._custom_dve(_OPS['SEED3'], out=t1, in0=u, s0=q[5], s1=q[4], imm2=q[3])
    nc.vector._custom_dve(_OPS['CHAIN3M'], out=t2, in0=u, in1=t1, s0=q[2], s1=q[1], imm2=q[0])
    nc.vector._custom_dve(_OPS['RELUADD'], out=out, in0=sb, in1=t2)


def _emit_g(nc, out, s, t1, t2):
    """out = ln(softplus(s)); s must be clamped to [-4.5, 4.5]."""
    g = [float(v) for v in _G_C]
    nc.vector._custom_dve(_OPS['SEED3'], out=t1, in0=s, s0=g[14], s1=g[13], imm2=g[12])
    a, b = t1, t2
    for k in (11, 8, 5):
        nc.vector._custom_dve(_OPS['CHAIN3'], out=b, in0=s, in1=a, s0=g[k], s1=g[k - 1], imm2=g[k - 2])
        a, b = b, a
    nc.vector._custom_dve(_OPS['CHAIN3'], out=out, in0=s, in1=a, s0=g[2], s1=g[1], imm2=g[0])


# ------------------------------------------------------------ host packing --
def _prep_weights(P):
    """Host-side weight/bias tensors, all float32 numpy."""
    f = lambda a: np.ascontiguousarray(np.asarray(a, np.float32))
    W = {}
    # phase 1
    W['wgt'] = f(P['gt'][0])                        # [10, 64] lhsT
    W['bgt'] = f(P['gt'][1]).reshape(64, 1)
    W['wez'] = f(P['ez'][0] * 0.5)                  # [64, 32]
    W['bez'] = f(P['ez'][1]).reshape(32, 1)
    # mu / sigma heads with bias folded in as a 33rd contraction row
    W['wmsq'] = f(np.vstack([np.asarray(P['mu'][0]), np.asarray(P['mu'][1])[None, :]]))
    W['wmss'] = f(np.vstack([np.asarray(P['sigma'][0]), np.asarray(P['sigma'][1])[None, :]]))

    # phase 2: block matmuls (pack-4, partition base 32j)
    def l1(w):  # input zprev: main (block j-1 -> j) + wrap (block 3 -> 0)
        w = np.asarray(w, np.float32)
        main = np.zeros((128, 106), np.float32)
        for j in (1, 2, 3):
            main[32 * (j - 1):32 * (j - 1) + 10, 32 * j:32 * j + 10] = w
        wrap = np.zeros((128, 10), np.float32)
        wrap[96:106, 0:10] = w
        return f(main), f(wrap)

    def diag(w, k_in=10):  # same-t blocks
        w = np.asarray(w, np.float32)
        d = np.zeros((106, 106), np.float32)
        for j in range(4):
            d[32 * j:32 * j + k_in, 32 * j:32 * j + w.shape[1]] = w
        return f(d)

    def bal(b):  # aligned bias row [1, 106] for bias-matmul
        b = np.asarray(b, np.float32)
        r = np.zeros((1, 106), np.float32)
        for j in range(4):
            r[0, 32 * j:32 * j + b.shape[0]] = b
        return f(r)

    W['nu1_m'], W['nu1_w'] = l1(P['nu1'][0]); W['nu1_b'] = bal(P['nu1'][1])
    W['mun1_m'], W['mun1_w'] = l1(P['mun1'][0]); W['mun1_b'] = bal(P['mun1'][1])
    W['mul_m'], W['mul_w'] = l1(P['mul'][0]); W['mul_b'] = bal(P['mul'][1])
    W['nu2_d'] = diag(P['nu2'][0]); W['nu2_nb'] = bal(P['nu2'][1])
    W['mun2_d'] = diag(P['mun2'][0]); W['mun2_b'] = bal(P['mun2'][1])
    W['dz_d'] = diag(P['dzsig'][0]); W['dz_b'] = bal(P['dzsig'][1])
    # iwae reduce vectors [106, 4]
    ind = np.zeros((106, 4), np.float32)
    for j in range(4):
        ind[32 * j:32 * j + 10, j] = 1.0
    W['red_h'] = f(ind * 0.5)
    W['red_nh'] = f(ind * -0.5)
    W['red_1'] = f(ind)
    W['ident'] = f(np.eye(128, dtype=np.float32))
    return W


def _pack_eps(eps_sh, T):
    """eps [B_SH, T, 10] -> aligned-sparse [128, (T//4)*64]."""
    G = T // 4
    out = np.zeros((128, G, B_SH), np.float32)
    e = eps_sh.transpose(1, 2, 0)  # [T, 10, B]
    for j in range(4):
        out[32 * j:32 * j + 10, :, :] = e[j::4].transpose(1, 0, 2)
    return np.ascontiguousarray(out.reshape(128, G * B_SH))


# ------------------------------------------------------------- the kernel ---
_CACHE = {}


def _build(T, Tc):
    """Build + finalize the per-core Bass program. Returns (nc, names)."""
    import os
    import concourse.bacc as bacc
    import concourse.mybir as mybir
    from concourse.tile import TileContext

    abl = set(os.environ.get('DKF_ABL', '').split(','))
    _register_ops()
    F32 = mybir.dt.float32
    AF = mybir.ActivationFunctionType
    ALU = mybir.AluOpType

    NCH = T // Tc              # chunks
    NG = Tc // 4               # col-groups per chunk
    NSUB = 4                   # xs sub-chunks per chunk
    SUBT = Tc // NSUB          # steps per xs sub-chunk
    SLAB = max(1, NG // 5) * 64  # phase-2 psum slab free size (320)
    NSL = NG * 64 // SLAB      # slabs per chunk (5)

    nc = bacc.Bacc("TRN2")
    dt = {}
    dt['xs'] = nc.dram_tensor("xs", [64, T * 64], F32, kind="ExternalInput")
    dt['eps'] = nc.dram_tensor("eps", [128, (T // 4) * 64], F32, kind="ExternalInput")
    dt['eps_f'] = nc.dram_tensor("eps_f", [10, T * 64], F32, kind="ExternalInput")
    wshapes = {
        'wgt': [10, 64], 'bgt': [64, 1], 'wez': [64, 32], 'bez': [32, 1],
        'wmsq': [33, 10], 'wmss': [33, 10],
        'nu1_m': [128, 106], 'nu1_w': [128, 10], 'nu1_b': [1, 106],
        'mun1_m': [128, 106], 'mun1_w': [128, 10], 'mun1_b': [1, 106],
        'mul_m': [128, 106], 'mul_w': [128, 10], 'mul_b': [1, 106],
        'nu2_d': [106, 106], 'nu2_nb': [1, 106],
        'mun2_d': [106, 106], 'mun2_b': [1, 106],
        'dz_d': [106, 106], 'dz_b': [1, 106],
        'red_h': [106, 4], 'red_nh': [106, 4], 'red_1': [106, 4],
        'ident': [128, 128],
    }
    for k, s in wshapes.items():
        dt[k] = nc.dram_tensor(k, s, F32, kind="ExternalInput")
    dt['z_out'] = nc.dram_tensor("z_out", [64, T * 10], F32, kind="ExternalOutput")
    dt['kl_out'] = nc.dram_tensor("kl_out", [64, T * 10], F32, kind="ExternalOutput")
    dt['iw_out'] = nc.dram_tensor("iw_out", [64, T], F32, kind="ExternalOutput")

    NCHAIN = int(os.environ.get('DKF_CHAINS', '1'))
    CWID = 64 // NCHAIN        # batch columns per chain

    OPS = _OPS
    with TileContext(nc) as tc:
        with tc.tile_pool(name="const", bufs=1) as cp, \
             tc.tile_pool(name="xs", bufs=3) as xsp, \
             tc.tile_pool(name="pk", bufs=2) as pkp, \
             tc.tile_pool(name="p1s", bufs=3) as p1s, \
             tc.tile_pool(name="p2c", bufs=8) as p2c, \
             tc.tile_pool(name="p2s", bufs=2) as p2s, \
             tc.tile_pool(name="stg", bufs=1) as stg, \
             tc.tile_pool(name="ps1", bufs=2, space="PSUM") as ps1, \
             tc.tile_pool(name="ps2", bufs=2, space="PSUM") as ps2, \
             tc.tile_pool(name="pst", bufs=1, space="PSUM") as pst:

            # constants
            w = {}
            for k, s in wshapes.items():
                w[k] = cp.tile(s, F32, tag=f"w_{k}", name=f"w_{k}")
                nc.sync.dma_start(w[k][:], dt[k][:])
            ones1 = cp.tile([1, SLAB], F32, tag="ones1")
            nc.vector.memset(ones1[:], 1.0)
            zinit = cp.tile([10, 64], F32, tag="zinit")
            nc.vector.memset(zinit[:], 0.0)

            # recurrence state: z enters the next gt matmul as two PSUM-
            # accumulated parts, Wgt.T@qmu + Wgt.T@(sigma*eps)
            prev_qm = [zinit[:, cc * CWID:(cc + 1) * CWID] for cc in range(NCHAIN)]
            prev_es = [zinit[:, cc * CWID:(cc + 1) * CWID] for cc in range(NCHAIN)]
            prev_zpk = None
            h_ones_set = [0] * NCHAIN
            q = [float(v) for v in _SP_C]

            for c in range(NCH):
                # --- chunk buffers
                xs_t = [xsp.tile([64, SUBT * 64], F32, tag="xs", name=f"xs_{c}_{i}")
                        for i in range(NSUB)]
                for si in range(NSUB):
                    off = (c * Tc + si * SUBT) * 64
                    nc.sync.dma_start(xs_t[si][:], dt['xs'][:, off:off + SUBT * 64])
                eps_pk = pkp.tile([128, NG * 64], F32, tag="eps")
                nc.sync.dma_start(eps_pk[:], dt['eps'][:, c * NG * 64:(c + 1) * NG * 64])
                eps_fl = [xsp.tile([10, SUBT * 64], F32, tag="epsf", name=f"epsf_{c}_{i}", bufs=2)
                          for i in range(NSUB)]
                for si in range(NSUB):
                    off = (c * Tc + si * SUBT) * 64
                    nc.sync.dma_start(eps_fl[si][:], dt['eps_f'][:, off:off + SUBT * 64])
                z_pk = pkp.tile([128, (NG + 1) * 64], F32, tag="zpk")
                sq_pk = pkp.tile([128, NG * 64], F32, tag="sqpk")
                sg_pk = pkp.tile([128, NG * 64], F32, tag="sgpk")
                if c < 2:
                    nc.vector.memset(z_pk[:], 0.0)
                    nc.vector.memset(sq_pk[:], 0.0)
                    nc.vector.memset(sg_pk[:], 1.0)
                if c == 0:
                    nc.vector.memset(z_pk[:, 0:64], 0.0)
                else:
                    nc.vector.tensor_copy(z_pk[:, 0:64], prev_zpk[:, NG * 64:(NG + 1) * 64])

                # --- phase 1: the serial scan (NCHAIN interleaved batch chains)
                for tl in range(Tc):
                    j, g = tl % 4, tl // 4
                    col = g * 64
                    pb = 32 * j
                    si, so = tl // SUBT, (tl % SUBT) * 64
                    for cc in range(NCHAIN):
                        bo = cc * CWID          # batch column offset
                        gt_ps = ps1.tile([64, CWID], F32, tag=f"g{cc}", bufs=1,
                                         name=f"gtps{cc}_{c}_{tl}")
                        nc.tensor.matmul(gt_ps[:], w['wgt'][:], prev_qm[cc], start=True, stop=False)
                        nc.tensor.matmul(gt_ps[:], w['wgt'][:], prev_es[cc], start=False, stop=True)
                        xt1 = p1s.tile([64, CWID], F32, tag=f"xt1{cc}", name=f"xt1{cc}_{c}_{tl}")
                        nc.scalar.activation(xt1[:], gt_ps[:], AF.Tanh, bias=w['bgt'][:, 0:1])
                        ez_ps = ps1.tile([32, CWID], F32, tag=f"e{cc}", bufs=2,
                                         name=f"ezps{cc}_{c}_{tl}")
                        nc.tensor.matmul(ez_ps[:], w['wez'][:],
                                         xs_t[si][:, so + bo:so + bo + CWID],
                                         start=True, stop=False)
                        nc.tensor.matmul(ez_ps[:], w['wez'][:], xt1[:], start=False, stop=True)
                        h = p1s.tile([33, CWID], F32, tag=f"h{cc}", name=f"h{cc}_{c}_{tl}")
                        if h_ones_set[cc] < 3:
                            nc.vector.memset(h[32:33, :], 1.0)
                            h_ones_set[cc] += 1
                        nc.scalar.activation(h[0:32, :], ez_ps[:], AF.Tanh,
                                             bias=w['bez'][:, 0:1])
                        qs_ps = ps1.tile([10, 2 * CWID], F32, tag=f"q{cc}", bufs=2,
                                         name=f"qsps{cc}_{c}_{tl}")
                        nc.tensor.matmul(qs_ps[:, CWID:2 * CWID], w['wmss'][:], h[:],
                                         start=True, stop=False)
                        nc.tensor.matmul(qs_ps[:, 0:CWID], w['wmsq'][:], h[:],
                                         start=False, stop=True, skip_group_check=True)
                        s_ap = qs_ps[0:10, CWID:2 * CWID]
                        sg_t = p1s.tile([10, CWID], F32, tag=f"sg{cc}", name=f"sg{cc}_{c}_{tl}")
                        if 'nosp' in abl:
                            nc.vector.tensor_copy(sg_t[:], s_ap)
                        else:
                            aa = p1s.tile([10, CWID], F32, tag=f"aa{cc}", name=f"aa{cc}_{c}_{tl}")
                            nc.scalar.activation(aa[:], s_ap, AF.Abs)
                            u = p1s.tile([10, CWID], F32, tag=f"u{cc}", name=f"u{cc}_{c}_{tl}")
                            nc.scalar.activation(u[:], aa[:], AF.Exp, scale=-1.0)
                            t1 = p1s.tile([10, CWID], F32, tag=f"t1{cc}", name=f"t1{cc}_{c}_{tl}")
                            t2 = p1s.tile([10, CWID], F32, tag=f"t2{cc}", name=f"t2{cc}_{c}_{tl}")
                            nc.vector._custom_dve(OPS['SEED3'], out=t1[:], in0=u[:], s0=q[5], s1=q[4], imm2=q[3])
                            nc.vector._custom_dve(OPS['CHAIN3M'], out=t2[:], in0=u[:], in1=t1[:], s0=q[2], s1=q[1], imm2=q[0])
                            nc.vector._custom_dve(OPS['RELUADD'], out=sg_t[:], in0=s_ap, in1=t2[:])
                        es = p1s.tile([10, CWID], F32, tag=f"es{cc}", name=f"es{cc}_{c}_{tl}")
                        nc.vector.tensor_tensor(es[:], sg_t[:],
                                                eps_fl[si][:, so + bo:so + bo + CWID], ALU.mult)
                        qm_t = p1s.tile([10, CWID], F32, tag=f"qm{cc}", name=f"qm{cc}_{c}_{tl}")
                        nc.vector.tensor_copy(qm_t[:], qs_ps[0:10, 0:CWID])
                        # z itself is only needed off the critical cycle
                        zh = p1s.tile([10, CWID], F32, tag=f"zh{cc}", name=f"zh{cc}_{c}_{tl}")
                        nc.vector.tensor_tensor(zh[:], qs_ps[0:10, 0:CWID], es[:], ALU.add)
                        if 'nocp' not in abl:
                            # packing copies (cross-base copies are safe)
                            nc.vector.tensor_copy(sg_pk[pb:pb + 10, col + bo:col + bo + CWID], sg_t[:])
                            nc.vector.tensor_copy(sq_pk[pb:pb + 10, col + bo:col + bo + CWID], s_ap)
                            nc.vector.tensor_copy(z_pk[pb:pb + 10, 64 + col + bo:64 + col + bo + CWID], zh[:])
                        prev_qm[cc] = qm_t
                        prev_es[cc] = es

                # --- phase 2 (wide) for this chunk
                if 'nop2' in abl:
                    prev_zpk = z_pk
                    continue
                CW = NG * 64

                def big(_c=c):
                    t = p2c.tile([128, CW], F32, tag="p2big")
                    if _c < 2:
                        nc.vector.memset(t[:], 0.0)
                    return t

                en, d_t, mulv, sbp = big(), big(), big(), big()
                for s in range(NSL):
                    S0 = s * SLAB
                    zc0 = 64 + S0            # main rhs offset in z_pk
                    p_nu1 = ps2.tile([106, SLAB], F32, tag="p2")
                    nc.tensor.matmul(p_nu1[:], w['nu1_m'][:], z_pk[:, zc0:zc0 + SLAB], start=True, stop=False)
                    nc.tensor.matmul(p_nu1[0:10, :], w['nu1_w'][:], z_pk[:, zc0 - 64:zc0 - 64 + SLAB], start=False, stop=False)
                    nc.tensor.matmul(p_nu1[:], w['nu1_b'][:], ones1[:, 0:SLAB], start=False, stop=True)
                    r1 = p2s.tile([128, SLAB], F32, tag="r1")
                    nc.scalar.activation(r1[0:106, :], p_nu1[:], AF.Relu)
                    p_nu2 = ps2.tile([106, SLAB], F32, tag="p2")
                    nc.tensor.matmul(p_nu2[:], w['nu2_d'][:], r1[0:106, :], start=True, stop=False)
                    nc.tensor.matmul(p_nu2[:], w['nu2_nb'][:], ones1[:, 0:SLAB], start=False, stop=True)
                    # en = exp(-(v + b)) ; nu2_nb already holds -b, psum scale -1
                    nc.scalar.activation(en[0:106, S0:S0 + SLAB], p_nu2[:], AF.Exp, scale=-1.0)
                    p_m1 = ps2.tile([106, SLAB], F32, tag="p2")
                    nc.tensor.matmul(p_m1[:], w['mun1_m'][:], z_pk[:, zc0:zc0 + SLAB], start=True, stop=False)
                    nc.tensor.matmul(p_m1[0:10, :], w['mun1_w'][:], z_pk[:, zc0 - 64:zc0 - 64 + SLAB], start=False, stop=False)
                    nc.tensor.matmul(p_m1[:], w['mun1_b'][:], ones1[:, 0:SLAB], start=False, stop=True)
                    r2 = p2s.tile([128, SLAB], F32, tag="r2")
                    nc.scalar.activation(r2[0:106, :], p_m1[:], AF.Relu)
                    p_m2 = ps2.tile([106, SLAB], F32, tag="p2")
                    nc.tensor.matmul(p_m2[:], w['mun2_d'][:], r2[0:106, :], start=True, stop=False)
                    nc.tensor.matmul(p_m2[:], w['mun2_b'][:], ones1[:, 0:SLAB], start=False, stop=True)
                    rmun = p2s.tile([128, SLAB], F32, tag="rmun")
                    nc.scalar.activation(rmun[0:106, :], p_m2[:], AF.Relu)
                    p_ml = ps2.tile([106, SLAB], F32, tag="p2")
                    nc.tensor.matmul(p_ml[:], w['mul_m'][:], z_pk[:, zc0:zc0 + SLAB], start=True, stop=False)
                    nc.tensor.matmul(p_ml[0:10, :], w['mul_w'][:], z_pk[:, zc0 - 64:zc0 - 64 + SLAB], start=False, stop=False)
                    nc.tensor.matmul(p_ml[:], w['mul_b'][:], ones1[:, 0:SLAB], start=False, stop=True)
                    nc.vector.tensor_copy(mulv[0:106, S0:S0 + SLAB], p_ml[:])
                    nc.vector.tensor_tensor(d_t[0:106, S0:S0 + SLAB], p_m2[:],
                                            mulv[0:106, S0:S0 + SLAB], ALU.subtract)
                    p_dz = ps2.tile([106, SLAB], F32, tag="p2")
                    nc.tensor.matmul(p_dz[:], w['dz_d'][:], rmun[0:106, :], start=True, stop=False)
                    nc.tensor.matmul(p_dz[:], w['dz_b'][:], ones1[:, 0:SLAB], start=False, stop=True)
                    nc.vector.tensor_copy(sbp[0:106, S0:S0 + SLAB], p_dz[:])

                nu = big()
                nc.vector.tensor_scalar(nu[:], en[:], 1.0, None, ALU.add)
                nc.vector.reciprocal_approx_fast(nu[:], nu[:])
                m1 = big()
                nc.vector.tensor_tensor(m1[:], nu[:], d_t[:], ALU.mult)
                pmu = big()
                nc.vector.tensor_tensor(pmu[:], mulv[:], m1[:], ALU.add)
                # sigma_p
                w1, w2 = big(), big()
                nc.vector._custom_dve(OPS['MINNEG'], out=w1[:], in0=sbp[:])
                up = big()
                nc.scalar.activation(up[:], w1[:], AF.Exp)
                sgp = big()
                _emit_softplus(nc, sgp[:], sbp[:], up[:], w1[:], w2[:])
                rp = big()
                nc.vector.reciprocal_approx_accurate(rp[:], sgp[:], w1[:])
                # L = g(sp) - g(sq)
                nc.vector.tensor_scalar(w1[:], sbp[:], 4.5, -4.5, ALU.min, ALU.max)
                gp = big()
                _emit_g(nc, gp[:], w1[:], w2[:], up[:])
                nc.vector.tensor_scalar(w1[:], sq_pk[:], 4.5, -4.5, ALU.min, ALU.max)
                gq = big()
                _emit_g(nc, gq[:], w1[:], w2[:], up[:])
                L = big()
                nc.vector.tensor_tensor(L[:], gp[:], gq[:], ALU.subtract)
                # iwae pieces
                dp = big()
                nc.vector.tensor_tensor(dp[:], z_pk[:, 64:64 + CW], pmu[:], ALU.subtract)
                nc.vector.tensor_tensor(dp[:], dp[:], rp[:], ALU.mult)
                sqp_t = big()
                nc.scalar.activation(sqp_t[:], dp[:], AF.Square)
                e2 = big()
                nc.scalar.activation(e2[:], eps_pk[:], AF.Square)
                # kl
                sq2 = big()
                nc.scalar.activation(sq2[:], sg_pk[:], AF.Square)
                qd = big()
                nc.vector.tensor_tensor(qd[:], sg_pk[:], eps_pk[:], ALU.mult)
                nc.vector.tensor_tensor(qd[:], z_pk[:, 64:64 + CW], qd[:], ALU.subtract)
                nc.vector.tensor_tensor(qd[:], qd[:], pmu[:], ALU.subtract)
                nc.scalar.activation(qd[:], qd[:], AF.Square)
                nc.vector.tensor_tensor(qd[:], qd[:], sq2[:], ALU.add)
                nc.vector.tensor_tensor(w2[:], rp[:], rp[:], ALU.mult)
                nc.vector.tensor_tensor(qd[:], qd[:], w2[:], ALU.mult)
                kl_t = big()
                nc.vector.affine_then_add(kl_t[:], qd[:], L[:], 0.5, -0.5)
                # iwae reduce
                iw_al = p2c.tile([4, CW], F32, tag="iwal", bufs=2)
                for s in range(NSL):
                    S0 = s * SLAB
                    p_iw = pst.tile([64, 512], F32, tag="pt")
                    nc.tensor.matmul(p_iw[0:4, 0:SLAB], w['red_h'][:], sqp_t[0:106, S0:S0 + SLAB], start=True, stop=False)
                    nc.tensor.matmul(p_iw[0:4, 0:SLAB], w['red_nh'][:], e2[0:106, S0:S0 + SLAB], start=False, stop=False)
                    nc.tensor.matmul(p_iw[0:4, 0:SLAB], w['red_1'][:], L[0:106, S0:S0 + SLAB], start=False, stop=True)
                    nc.vector.tensor_copy(iw_al[:, S0:S0 + SLAB], p_iw[0:4, 0:SLAB])
                # transposes to batch-major
                z_st = stg.tile([64, NG * 40], F32, tag="zst")
                kl_st = stg.tile([64, NG * 40], F32, tag="klst")
                iw_st = stg.tile([64, NG * 4], F32, tag="iwst")
                for g in range(NG):
                    pt = pst.tile([64, 512], F32, tag="pt")
                    nc.tensor.matmul(pt[:, 0:128], z_pk[:, 64 + g * 64:128 + g * 64], w['ident'][:], is_transpose=True)
                    src = pt[:, 0:128].rearrange("p (j d) -> p j d", j=4)[:, :, 0:10]
                    nc.vector.tensor_copy(z_st[:, g * 40:(g + 1) * 40].rearrange("p (j d) -> p j d", j=4), src)
                    pt2 = pst.tile([64, 512], F32, tag="pt")
                    nc.tensor.matmul(pt2[:, 0:128], kl_t[:, g * 64:(g + 1) * 64], w['ident'][:], is_transpose=True)
                    src2 = pt2[:, 0:128].rearrange("p (j d) -> p j d", j=4)[:, :, 0:10]
                    nc.vector.tensor_copy(kl_st[:, g * 40:(g + 1) * 40].rearrange("p (j d) -> p j d", j=4), src2)
                    pt3 = pst.tile([64, 512], F32, tag="pt")
                    nc.tensor.matmul(pt3[:, 0:4], iw_al[:, g * 64:(g + 1) * 64], w['ident'][0:4, 0:4], is_transpose=True)
                    nc.vector.tensor_copy(iw_st[:, g * 4:(g + 1) * 4], pt3[:, 0:4])
                nc.sync.dma_start(dt['z_out'][:, c * Tc * 10:(c + 1) * Tc * 10], z_st[:])
                nc.sync.dma_start(dt['kl_out'][:, c * Tc * 10:(c + 1) * Tc * 10], kl_st[:])
                nc.sync.dma_start(dt['iw_out'][:, c * Tc:(c + 1) * Tc], iw_st[:])
                prev_zpk = z_pk

    nc.finalize()
    return nc


def kernel(xs, eps, mask, params):
    xs = np.asarray(xs, np.float32)
    eps = np.asarray(eps, np.float32)
    mask = np.asarray(mask).astype(bool)
    P = {k: (np.asarray(v[0], np.float32), np.asarray(v[1], np.float32))
         for k, v in params.items()}
    if not mask.all():
        return _numpy_ref(xs, eps, mask, P)

    B, T, _ = xs.shape
    Tc = T if T <= 100 else 100
    assert T % Tc == 0 and Tc % 4 == 0
    key = (T, Tc)
    if key not in _CACHE:
        _CACHE[key] = _build(T, Tc)
    nc = _CACHE[key]

    from concourse.bass_utils import run_bass_kernel_spmd
    W = _prep_weights(P)
    in_maps = []
    ncores = B // B_SH
    for c in range(ncores):
        sl = slice(c * B_SH, (c + 1) * B_SH)
        xs_fm = np.ascontiguousarray(xs[sl].transpose(2, 1, 0)).reshape(64, T * 64)
        eps_f = np.ascontiguousarray(eps[sl].transpose(2, 1, 0)).reshape(10, T * 64)
        m = {"xs": xs_fm, "eps": _pack_eps(eps[sl], T), "eps_f": eps_f}
        m.update(W)
        in_maps.append(m)
    res = run_bass_kernel_spmd(nc, in_maps, core_ids=list(range(ncores)))
    z = np.empty((B, T, 10), np.float32)
    kl = np.empty((B, T, 10), np.float32)
    iw = np.empty((B, T), np.float32)
    for c in range(ncores):
        r = res.results[c]
        sl = slice(c * B_SH, (c + 1) * B_SH)
        z[sl] = r["z_out"].reshape(B_SH, T, 10)
        kl[sl] = r["kl_out"].reshape(B_SH, T, 10)
        iw[sl] = r["iw_out"].reshape(B_SH, T)
    return z, kl, iw
